# revision 1
# baseline (speedup 1.0000x reference)
"""Trainium2 Bass kernel for NeuralTensorLayer (order-1/2/3 polynomial layer).

    out[b,l] = bias[l] + sum_i X[b,i] W1[i,l]
             + sum_ij X[b,i] X[b,j] W2[i,j,l]
             + sum_ijk X[b,i] X[b,j] X[b,k] W3[i,j,k,l]

with B=32768, D=K=32, data-parallel over 8 NeuronCores (4096 rows each).

Strategy (per core):
  * Exploit (i,j) symmetry: only the 528 pairs i<=j are needed against
    host-symmetrized weights W3s[ij,k,l] = W3[i,j,k,l]+W3[j,i,k,l] (i<j),
    cutting the dominant matmul contraction from 1024 -> 528 (+32 X rows).
  * Pair operands arrive host-pregathered per supertile (8 batch tiles) in
    one DRAM block XX[s] = [128, 5x1024 XE | 5x1024 XR] (bf16); the DVE
    builds Z^T[p,b] = X_i X_j (bf16) chunk by chunk, interleaved with the
    per-tile post-processing so no queue ever head-of-line blocks.
  * One matmul group per 128-row tile accumulating into two PSUM tiles:
    big [128,1024] (T3, l-major k) and low [128,64] (out_low = W2s/W1).
    big pool is triple-buffered so the PE never waits on evacuation.
  * Dummy matmuls at kernel start keep the PE busy while the first input
    DMAs land, tripping the HAM clock-gate to 2.4 GHz early.  Startup DMAs
    are issued in need-time order round-robin across the sync/scalar/
    gpsimd queues (each queue's ring serializes issue+transfer).
  * Post per tile: ScalarE copies PSUM->SBUF bf16 into an interleaved
    (l, 34) layout [k<32 big | out_low | junk], DVE multiplies by the
    Xext broadcast (2x mode; the junk column is killed by the 0.0 pad)
    and reduce-sums k (1x) into a per-supertile [128,256] tile written
    back with a single dense DMA.  bias added on host.
"""

import numpy as np
import ml_dtypes
from contextlib import ExitStack

import concourse.bass as bass
import concourse.bacc as bacc
import concourse.tile as tile
from concourse import mybir
from concourse import bass_utils

BF16 = ml_dtypes.bfloat16

B, D, KOUT = 32768, 32, 32
NCORES = 8
BLOC = B // NCORES          # 4096 rows per core
P = 128                     # rows per tile
SUPER = 8                   # tiles per supertile
SP = SUPER * P              # 1024
NSUPER = BLOC // SP         # 4
NPAIRS = D * (D + 1) // 2   # 528
CHUNK_P = [128, 128, 128, 128, 48]  # matmul partitions per chunk
KG = 34                     # k-grid width: 32 order-3 + out_low + junk
NBIG = 1024                 # big psum columns (l*32+k, k<32)
NCOL = KOUT * KG            # 1088 staged columns
XW = D + 2                  # host-padded X width: 32 + [1.0, 0.0]
NDUMMY = 6                  # PE warm-up matmuls (HAM un-throttle)
MINI = 2 * P                # early columns per chunk for tiles 0-1

PAIRS = [(i, j) for i in range(D) for j in range(i, D)]
I_P = np.array([p[0] for p in PAIRS], np.int32)
J_P = np.array([p[1] for p in PAIRS], np.int32)

F32 = mybir.dt.float32
BF = mybir.dt.bfloat16


def _pack_weights(W1, W2, W3):
    W1 = np.asarray(W1, np.float64)
    W2 = np.asarray(W2, np.float64)
    W3 = np.asarray(W3, np.float64)
    Wcat = np.zeros((5, 128, KOUT, KG), np.float64)
    for p, (i, j) in enumerate(PAIRS):
        c, pp = divmod(p, 128)
        if i < j:
            w3 = W3[i, j] + W3[j, i]   # [k, l]
            w2 = W2[i, j] + W2[j, i]   # [l]
        else:
            w3 = W3[i, i]
            w2 = W2[i, i]
        Wcat[c, pp, :, :D] = w3.T
        Wcat[c, pp, :, D] = w2
    for dd in range(D):                # order-1: X rows in chunk 4
        Wcat[4, 16 + dd, :, D] = W1[dd]
    # big part (l-major k, col l*32+k) then low part (out_low col l, pad 0)
    big = Wcat[:, :, :, :D].reshape(5, 128, KOUT * D)
    low = np.zeros((5, 128, 64), np.float64)
    low[:, :, :KOUT] = Wcat[:, :, :, D]
    packed = np.concatenate([big, low], axis=2)  # [5, 128, 1088]
    return packed.astype(np.float32).astype(BF16)


def _build_module():
    nc = bacc.Bacc("TRN2", target_bir_lowering=False, debug=False,
                   enable_asserts=False)
    XXd = nc.dram_tensor("XX", [NSUPER, 128, 10 * SP], BF, kind="ExternalInput").ap()
    XBDd = nc.dram_tensor("XBD", [NSUPER, 128, SUPER * XW], BF, kind="ExternalInput").ap()
    XTd = nc.dram_tensor("XT", [D, BLOC], BF, kind="ExternalInput").ap()
    WCd = nc.dram_tensor("WCAT", [5, 128, NCOL], BF, kind="ExternalInput").ap()
    OUTd = nc.dram_tensor("OUT", [NSUPER, 128, SUPER * KOUT], F32, kind="ExternalOutput").ap()

    XE_OFF = [c * SP for c in range(5)]
    XR_OFF = [5 * SP + c * SP for c in range(5)]

    with ExitStack() as ctx:
        tc = ctx.enter_context(tile.TileContext(nc))
        consts = ctx.enter_context(tc.tile_pool(name="consts", bufs=1))
        xxpool = ctx.enter_context(tc.tile_pool(name="xxpool", bufs=3))
        xbpool = ctx.enter_context(tc.tile_pool(name="xbpool", bufs=3))
        zpool = ctx.enter_context(tc.tile_pool(name="zpool", bufs=2))
        spool = ctx.enter_context(tc.tile_pool(name="spool", bufs=6))
        upool = ctx.enter_context(tc.tile_pool(name="upool", bufs=4))
        opool = ctx.enter_context(tc.tile_pool(name="opool", bufs=2))
        bigps = ctx.enter_context(tc.tile_pool(name="bigps", bufs=3, space="PSUM"))
        lowps = ctx.enter_context(tc.tile_pool(name="lowps", bufs=2, space="PSUM"))

        g = consts.tile([128, 640], BF, tag="g")
        nc.gpsimd.memset(g, 0.0)

        w_sb = []
        for c in range(5):
            w = consts.tile([128, NCOL], BF, tag=f"w_{c}")
            w_sb.append(w)
        xx_tiles = {0: xxpool.tile([128, 10 * SP], BF, tag="xx", name="xx0")}
        xbd = [None] * NSUPER
        xbd[0] = xbpool.tile([128, SUPER * XW], BF, tag="xbd", name="xbd0")

        # ---- startup DMAs: sync carries the weights-c0 + XE stream, the
        # gpsimd (SWDGE) queue carries w2/w4 + the XR stream, scalar only
        # w1/w3 (+XT rows) so its queue stays clear for the evacuations.
        xx0 = xx_tiles[0]
        nc.sync.dma_start(out=w_sb[0], in_=WCd[0])
        nc.scalar.dma_start(out=w_sb[1], in_=WCd[1])
        nc.gpsimd.dma_start(out=w_sb[2], in_=WCd[2])
        nc.gpsimd.dma_start(out=w_sb[4], in_=WCd[4])
        for c in range(5):
            e0, r0 = XE_OFF[c], XR_OFF[c]
            nc.sync.dma_start(out=xx0[:, e0:e0 + MINI], in_=XXd[0][:, e0:e0 + MINI])
            nc.gpsimd.dma_start(out=xx0[:, r0:r0 + MINI], in_=XXd[0][:, r0:r0 + MINI])
        nc.scalar.dma_start(out=w_sb[3], in_=WCd[3])
        for c in range(5):
            e0, r0 = XE_OFF[c] + MINI, XR_OFF[c] + MINI
            nc.sync.dma_start(out=xx0[:, e0:e0 + SP - MINI],
                              in_=XXd[0][:, e0:e0 + SP - MINI])
            nc.gpsimd.dma_start(out=xx0[:, r0:r0 + SP - MINI],
                                in_=XXd[0][:, r0:r0 + SP - MINI])
        nc.sync.dma_start(out=xbd[0], in_=XBDd[0])

        # PE warm-up: results are discarded (start=True clears the bank for
        # the first real accumulation into the same buffers later).
        for _ in range(NDUMMY):
            dummy = bigps.tile([128, NBIG], F32, tag="big")
            nc.tensor.matmul(dummy[:, 0:512], g[:, :128], g[:, 128:640],
                             start=True, stop=True)

        def z_tiles(sfx):
            zs = []
            for c in range(5):
                z = zpool.tile([CHUNK_P[c], SP], BF, tag=f"z{c}", name=f"z{c}{sfx}")
                zs.append(z)
            return zs

        def z_mul(s, zs, c, a, b):
            """pair products for column range [a,b) of chunk c (vector)."""
            xx = xx_tiles[s]
            pc = 128 if c < 4 else 16
            nc.vector.tensor_mul(zs[c][:pc, a:b],
                                 xx[:pc, XE_OFF[c] + a:XE_OFF[c] + b],
                                 xx[:pc, XR_OFF[c] + a:XR_OFF[c] + b])

        def z_xrows(s, zs, a, b):
            """order-1 X rows -> partitions 16:48 of chunk 4 (after z_mul)."""
            nc.scalar.dma_start(out=zs[4][16:48, a:b],
                                in_=XTd[:, s * SP + a:s * SP + b])

        def fetch_xe(s):
            """sync-queue burst: XE chunks + XBD for supertile s."""
            xx = xxpool.tile([128, 10 * SP], BF, tag="xx", name=f"xx{s}")
            xx_tiles[s] = xx
            for c in range(5):
                e0 = XE_OFF[c]
                nc.sync.dma_start(out=xx[:, e0:e0 + SP], in_=XXd[s][:, e0:e0 + SP])
            xbd[s] = xbpool.tile([128, SUPER * XW], BF, tag="xbd", name=f"xbd{s}")
            nc.sync.dma_start(out=xbd[s], in_=XBDd[s])

        def fetch_xr_chunk(s, c):
            """one XR chunk via the gpsimd (SWDGE) queue, keeping the scalar
            queue free for the PSUM evacuations."""
            xx = xx_tiles[s]
            r0 = XR_OFF[c]
            nc.gpsimd.dma_start(out=xx[:, r0:r0 + SP], in_=XXd[s][:, r0:r0 + SP])

        # z for supertile 0: tiles 0-1 columns now, the rest interleaved
        # into the tile-0/1 post-ops below as its DMAs land.
        zs0 = z_tiles("s0")
        for c in range(5):
            z_mul(0, zs0, c, 0, MINI)
        z_xrows(0, zs0, 0, MINI)
        # supertile 1 inputs: XE burst on sync, XR chunks via gpsimd
        if NSUPER > 1:
            fetch_xe(1)
            for c in range(5):
                fetch_xr_chunk(1, c)
        zs_cur = zs0

        for s in range(NSUPER):
            zs_next = z_tiles(f"s{s + 1}") if s + 1 < NSUPER else None
            osb = opool.tile([128, SUPER * KOUT], F32, tag="osb")
            for t in range(SUPER):
                big = bigps.tile([128, NBIG], F32, tag="big")
                low = lowps.tile([128, 64], F32, tag="low")
                for c in range(5):
                    pcp = CHUNK_P[c]
                    st = zs_cur[c][:pcp, t * P:(t + 1) * P]
                    first, last = c == 0, c == 4
                    nc.tensor.matmul(big[:, 0:512], st, w_sb[c][:pcp, 0:512],
                                     start=first, stop=last)
                    nc.tensor.matmul(big[:, 512:1024], st, w_sb[c][:pcp, 512:1024],
                                     start=first, stop=last)
                    nc.tensor.matmul(low, st, w_sb[c][:pcp, 1024:1088],
                                     start=first, stop=last)
                # staged layout is (l, 34): k<32 from big, k=32 out_low,
                # k=33 junk (killed by the 0.0 pad in xbd).
                staged2 = spool.tile([128, NCOL], BF, tag="staged2")
                stv = staged2[:, :].rearrange("p (l k) -> p l k", k=KG)
                nc.scalar.copy(out=stv[:, :, D:KG],
                               in_=low[:, :].rearrange("p (k l) -> p l k", k=2))
                nc.scalar.copy(out=stv[:, :, 0:D],
                               in_=big[:, :].rearrange("p (l k) -> p l k", k=D))
                u = upool.tile([128, NCOL], BF, tag="u")
                xk = (xbd[s][:, t * XW:(t + 1) * XW]
                      .unsqueeze(1).broadcast_to([P, KOUT, XW]))
                nc.vector.tensor_mul(
                    u[:, :].rearrange("p (l k) -> p l k", k=KG), stv, xk)
                nc.vector.reduce_sum(
                    out=osb[:, t * KOUT:(t + 1) * KOUT],
                    in_=u[:, :].rearrange("p (l k) -> p l k", k=KG),
                    axis=mybir.AxisListType.X)
                # supertile 0 only: build the remaining z columns as the
                # rest DMAs land, without blocking the post-op stream.
                if s == 0 and t <= 1:
                    for c in (0, 1) if t == 0 else (2, 3, 4):
                        z_mul(0, zs0, c, MINI, SP)
                    if t == 1:
                        z_xrows(0, zs0, MINI, SP)
                # spread next-supertile work through this supertile's tiles:
                # XE burst + XR chunks on the DMA queues, z products on the
                # vector queue once their inputs have landed.
                if s + 2 < NSUPER:
                    if t == 0:
                        fetch_xe(s + 2)
                    if t <= 4:
                        fetch_xr_chunk(s + 2, t)
                if zs_next is not None and 2 <= t <= 6:
                    z_mul(s + 1, zs_next, t - 2, 0, SP)
                    if t == 6:
                        z_xrows(s + 1, zs_next, 0, SP)
            nc.sync.dma_start(out=OUTd[s], in_=osb)
            zs_cur = zs_next
    nc.compile()
    return nc


_CACHE = {}


def _get_module():
    if "nc" not in _CACHE:
        _CACHE["nc"] = _build_module()
    return _CACHE["nc"]


def kernel(X, W1, W2, W3, bias):
    X = np.ascontiguousarray(np.asarray(X, np.float32))
    bias = np.asarray(bias, np.float32)
    Wcat = _pack_weights(W1, W2, W3)

    nc = _get_module()
    Xb = X.astype(BF16)                      # [B, D] bf16 (single rounding point)
    XbT = np.ascontiguousarray(Xb.T)         # [D, B] bf16
    npad = 5 * 128 - NPAIRS
    XE = np.concatenate([XbT[I_P], np.zeros((npad, B), BF16)], 0).reshape(5, 128, B)
    XR = np.concatenate([XbT[J_P], np.zeros((npad, B), BF16)], 0).reshape(5, 128, B)
    Xpad = np.zeros((B, XW), BF16)
    Xpad[:, :D] = Xb
    Xpad[:, D] = BF16(1.0)

    in_maps = []
    for c in range(NCORES):
        lo, hi = c * BLOC, (c + 1) * BLOC
        xe = (XE[:, :, lo:hi].reshape(5, 128, NSUPER, SP)
              .transpose(2, 1, 0, 3).reshape(NSUPER, 128, 5 * SP))
        xr = (XR[:, :, lo:hi].reshape(5, 128, NSUPER, SP)
              .transpose(2, 1, 0, 3).reshape(NSUPER, 128, 5 * SP))
        xx = np.concatenate([xe, xr], axis=2)        # [NSUPER, 128, 10*SP]
        xbd = (Xpad[lo:hi].reshape(NSUPER, SUPER, P, XW)
               .transpose(0, 2, 1, 3).reshape(NSUPER, 128, SUPER * XW))
        in_maps.append({
            "XX": np.ascontiguousarray(xx),
            "XBD": np.ascontiguousarray(xbd),
            "XT": np.ascontiguousarray(XbT[:, lo:hi]),
            "WCAT": Wcat,
        })
    res = bass_utils.run_bass_kernel_spmd(nc, in_maps, core_ids=list(range(NCORES)))
    _CACHE["last_results"] = res
    outs = []
    for c in range(NCORES):
        od = np.asarray(res.results[c]["OUT"])       # [NSUPER, 128, SUPER*KOUT]
        outs.append(od.reshape(NSUPER, P, SUPER, KOUT)
                    .transpose(0, 2, 1, 3).reshape(BLOC, KOUT))
    out = np.concatenate(outs, 0)
    return (out + bias.reshape(1, KOUT)).astype(np.float32)



# revision 8
# speedup vs baseline: 1.2739x; 1.2739x over previous
"""Trainium2 Bass kernel for NeuralTensorLayer (order-1/2/3 polynomial layer).

    out[b,l] = bias[l] + sum_i X[b,i] W1[i,l]
             + sum_ij X[b,i] X[b,j] W2[i,j,l]
             + sum_ijk X[b,i] X[b,j] X[b,k] W3[i,j,k,l]

B=32768, D=K=32, data-parallel over 8 NeuronCores (4096 rows each).

v2 strategy:
  * Full (i,j,k) symmetrization: group the order-3 sum by sorted triple
    (a<=b<=c) so  out3 = sum_{a<=b} Z_ab * sum_{c>=b} X_c W3f[(a,b),c,l]
    with W3f summing all distinct permutations.  Sorting the 528 pairs by
    b (the larger index) makes W3f's k-support a suffix [j0_chunk, 32) per
    128-pair chunk: widths [32,17,10,5,1] -> only 2080 big-matmul columns
    per 128-row tile instead of 5*1024 (the 5984 unique triples, vs 16896
    streamed MACs before).  The last 16 pairs (j=31, k=31 only) ride in
    the low matmul; their X_31 scaling is applied by ScalarE during PSUM
    evacuation (activation scale = per-partition X_31).
  * Z (pair products, fp16) are precomputed on host and DMA'd in directly
    - no on-device pair-product build, and much less DMA than shipping
    gathered XE/XR streams.
  * Whole pipeline in fp16 (PE speed identical to bf16, 8x less rounding
    error; all values comfortably in fp16 range).
  * Stage 2 split across engines so the PE (~1.2us/tile) is the only
    near-critical engine: ScalarE evacuates PSUM (big copy + low copy +
    X31-scaled copy), DVE does the X_k broadcast multiply and the final
    18-wide reduce, GpSimd does the 16->8 fold add.
  * PSUM: big [128,1024] (l-major k) triple buffered + low [128,64]
    double buffered = 8 banks.  Dummy matmuls at start warm the HAM
    clock gate while the first DMAs land.
"""

import numpy as np
import ml_dtypes
from contextlib import ExitStack

import concourse.bass as bass
import concourse.bacc as bacc
import concourse.tile as tile
from concourse import mybir
from concourse import bass_utils

FP16 = np.float16

B, D, KOUT = 32768, 32, 32
NCORES = 8
BLOC = B // NCORES          # 4096 rows per core
P = 128                     # rows per tile
SUPER = 8                   # tiles per supertile
SP = SUPER * P              # 1024
NSUPER = BLOC // SP         # 4
NDUMMY = 3                  # PE warm-up matmuls (HAM un-throttle)

# pairs (i,j), i<=j, sorted by j then i: p = j(j+1)/2 + i
PAIRS = [(i, j) for j in range(D) for i in range(j + 1)]
NPAIRS = len(PAIRS)         # 528
I_P = np.array([p[0] for p in PAIRS], np.int64)
J_P = np.array([p[1] for p in PAIRS], np.int64)
J0 = [int(J_P[128 * c]) for c in range(4)]      # [0, 15, 22, 27]
W_C = [D - j for j in J0]                       # k-window widths [32,17,10,5]

F32 = mybir.dt.float32
F16 = mybir.dt.float16


def _pack_weights(W1, W2, W3):
    """Returns [w0..w3] ([128, 32*w+32] fp16) and w4 ([48, 64] fp16)."""
    W1 = np.asarray(W1, np.float64)
    W2 = np.asarray(W2, np.float64)
    W3 = np.asarray(W3, np.float64)
    from itertools import permutations
    S6 = np.zeros((D, D, D, KOUT))
    for perm in set(permutations((0, 1, 2))):
        S6 += np.transpose(W3, perm + (3,))
    # W3f[p, c, l], zero for c < j(p); multiplicity fix for repeated indices
    W3f = np.zeros((NPAIRS, D, KOUT))
    for p, (a, bb) in enumerate(PAIRS):
        for c in range(bb, D):
            if a == bb == c:
                f = 1.0 / 6.0
            elif a == bb or bb == c:
                f = 0.5
            else:
                f = 1.0
            W3f[p, c] = S6[a, bb, c] * f
    W2s = np.empty((NPAIRS, KOUT))
    for p, (a, bb) in enumerate(PAIRS):
        W2s[p] = W2[a, bb] + W2[bb, a] if a < bb else W2[a, a]

    ws = []
    for c in range(4):
        j0, w = J0[c], W_C[c]
        # chunk 0 gets a 64-wide low block (cols 32:64 zero) so its
        # start=True matmul claims the full low PSUM range
        loww = 64 if c == 0 else 32
        wt = np.zeros((128, 32 * w + loww))
        blk = W3f[128 * c:128 * (c + 1), j0:, :]        # [128, w(k), 32(l)]
        blk = np.transpose(blk, (0, 2, 1)).reshape(128, KOUT * w)  # (l,k)
        wt[:, :16 * w] = blk[:, :16 * w]
        wt[:, 16 * w:32 * w] = blk[:, 16 * w:]
        wt[:, 32 * w:32 * w + 32] = W2s[128 * c:128 * (c + 1)]
        ws.append(wt.astype(np.float32).astype(FP16))
    w4 = np.zeros((48, 64))
    w4[:16, :KOUT] = W2s[512:]
    w4[16:, :KOUT] = W1
    w4[:16, KOUT:] = W3f[512:, 31, :]
    ws.append(w4.astype(np.float32).astype(FP16))
    return ws


def _build_module():
    nc = bacc.Bacc("TRN2", target_bir_lowering=False, debug=False,
                   enable_asserts=False)
    ZZd = nc.dram_tensor("ZZ", [NSUPER, 128, 8 * 512], F16, kind="ExternalInput").ap()
    Z4d = nc.dram_tensor("Z4", [NSUPER, 48, SP], F16, kind="ExternalInput").ap()
    XBDd = nc.dram_tensor("XBD", [NSUPER, 128, SUPER * KOUT], F16, kind="ExternalInput").ap()
    X31d = nc.dram_tensor("X31", [NSUPER, 128, SUPER], F32, kind="ExternalInput").ap()
    Wd = [nc.dram_tensor(f"W{c}", [128, 32 * W_C[c] + (64 if c == 0 else 32)], F16,
                         kind="ExternalInput").ap()
          for c in range(4)]
    W4d = nc.dram_tensor("W4", [48, 64], F16, kind="ExternalInput").ap()
    OUTd = nc.dram_tensor("OUT", [NSUPER, 128, SUPER * KOUT], F32, kind="ExternalOutput").ap()

    with ExitStack() as ctx:
        tc = ctx.enter_context(tile.TileContext(nc))
        consts = ctx.enter_context(tc.tile_pool(name="consts", bufs=1))
        zzpool = ctx.enter_context(tc.tile_pool(name="zzpool", bufs=2))
        z4pool = ctx.enter_context(tc.tile_pool(name="z4pool", bufs=2))
        xbpool = ctx.enter_context(tc.tile_pool(name="xbpool", bufs=2))
        spool = ctx.enter_context(tc.tile_pool(name="spool", bufs=3))
        upool = ctx.enter_context(tc.tile_pool(name="upool", bufs=3))
        vpool = ctx.enter_context(tc.tile_pool(name="vpool", bufs=3))
        opool = ctx.enter_context(tc.tile_pool(name="opool", bufs=2))
        bigps = ctx.enter_context(tc.tile_pool(name="bigps", bufs=3, space="PSUM"))
        lowps = ctx.enter_context(tc.tile_pool(name="lowps", bufs=2, space="PSUM"))

        g = consts.tile([128, 640], F16, tag="g")
        nc.vector.memset(g, 0.0)

        w_sb = [consts.tile([128, 32 * W_C[c] + (64 if c == 0 else 32)], F16,
                            tag=f"w_{c}", name=f"w_{c}")
                for c in range(4)]
        w4_sb = consts.tile([48, 64], F16, tag="w4")

        zz = {}
        z4 = {}
        xbd = {}
        x31 = {}

        def fetch_zz(s, eng):
            zt = zzpool.tile([128, 8 * 512], F16, tag="zz", name=f"zz{s}")
            zz[s] = zt
            if s == 0:
                eng.dma_start(out=zt[:, 0:512], in_=ZZd[0][:, 0:512])
                eng.dma_start(out=zt[:, 512:1024], in_=ZZd[0][:, 512:1024])
                eng.dma_start(out=zt[:, 1024:2048], in_=ZZd[0][:, 1024:2048])
                eng.dma_start(out=zt[:, 2048:4096], in_=ZZd[0][:, 2048:4096])

        def fetch_zz_part(s, part, eng):
            lo, hi = part * 1024, (part + 1) * 1024
            eng.dma_start(out=zz[s][:, lo:hi], in_=ZZd[s][:, lo:hi])

        def fetch_z4_xbd(s, eng):
            z4t = z4pool.tile([48, SP], F16, tag="z4", name=f"z4_{s}")
            z4[s] = z4t
            eng.dma_start(out=z4t, in_=Z4d[s])
            xt = xbpool.tile([128, SUPER * KOUT], F16, tag="xbd", name=f"xbd{s}")
            xbd[s] = xt
            eng.dma_start(out=xt, in_=XBDd[s])
            x31t = xbpool.tile([128, SUPER], F32, tag="x31", name=f"x31_{s}")
            x31[s] = x31t
            eng.dma_start(out=x31t, in_=X31d[s])

        # ---- startup DMAs
        nc.scalar.dma_start(out=w_sb[0], in_=Wd[0])
        fetch_zz(0, nc.sync)
        nc.scalar.dma_start(out=w_sb[1], in_=Wd[1])
        nc.gpsimd.dma_start(out=w4_sb, in_=W4d)
        fetch_z4_xbd(0, nc.gpsimd)
        nc.scalar.dma_start(out=w_sb[2], in_=Wd[2])
        nc.scalar.dma_start(out=w_sb[3], in_=Wd[3])

        # PE warm-up (results discarded; tiles recycled by the pool)
        for _ in range(NDUMMY):
            dummy = bigps.tile([128, 1024], F32, tag="big")
            nc.tensor.matmul(dummy[:, 0:512], g[:, :128], g[:, 128:640],
                             start=True, stop=True)

        for s in range(NSUPER):
            osb = opool.tile([128, SUPER * KOUT], F32, tag="osb")
            for t in range(SUPER):
                big = bigps.tile([128, 1024], F32, tag="big")
                low = lowps.tile([128, 64], F32, tag="low")
                bigv = big[:, :].rearrange("p (l k) -> p l k", k=D)
                z0 = zz[s][:, t * 512:(t + 1) * 512]
                # chunk 0: full k window
                nc.tensor.matmul(big[:, 0:512], z0[:, 0:128], w_sb[0][:, 0:512],
                                 start=True, stop=False)
                nc.tensor.matmul(big[:, 512:1024], z0[:, 0:128], w_sb[0][:, 512:1024],
                                 start=True, stop=False)
                nc.tensor.matmul(low[:, 0:64], z0[:, 0:128], w_sb[0][:, 1024:1088],
                                 start=True, stop=False)
                # chunks 1-3: suffix k windows, strided PSUM writes
                for c in (1, 2, 3):
                    j0, w = J0[c], W_C[c]
                    zc = zz[s][:, t * 512 + c * 128: t * 512 + (c + 1) * 128]
                    last = c == 3
                    nc.tensor.matmul(bigv[:, 0:16, j0:D], zc, w_sb[c][:, 0:16 * w],
                                     start=False, stop=last)
                    nc.tensor.matmul(bigv[:, 16:32, j0:D], zc, w_sb[c][:, 16 * w:32 * w],
                                     start=False, stop=last)
                    nc.tensor.matmul(low[:, 0:32], zc, w_sb[c][:, 32 * w:32 * w + 32],
                                     start=False, stop=False)
                # chunk 4: 16 pairs (k=31 only) + X rows, merged into low
                z4t = z4[s][:, t * 128:(t + 1) * 128]
                nc.tensor.matmul(low[:, 0:64], z4t, w4_sb,
                                 start=False, stop=True)

                # ---- stage 2
                staged = spool.tile([128, 1024], F16, tag="staged")
                nc.scalar.copy(out=staged, in_=big)
                v = vpool.tile([128, 32 * 18], F16, tag="v")
                v3 = v[:, :].rearrange("p (l c) -> p l c", c=18)
                nc.scalar.copy(out=v3[:, :, 16:17], in_=low[:, 0:32].unsqueeze(2))
                nc.scalar.mul(v3[:, :, 17:18], low[:, 32:64].unsqueeze(2),
                              x31[s][:, t:t + 1])
                u = upool.tile([128, 1024], F16, tag="u")
                u3 = u[:, :].rearrange("p (l k) -> p l k", k=D)
                st3 = staged[:, :].rearrange("p (l k) -> p l k", k=D)
                xk = (xbd[s][:, t * KOUT:(t + 1) * KOUT]
                      .unsqueeze(1).broadcast_to([P, KOUT, D]))
                nc.vector.tensor_mul(u3, st3, xk)
                nc.gpsimd.tensor_add(v3[:, :, 0:16], u3[:, :, 0:16], u3[:, :, 16:32])
                nc.vector.reduce_sum(out=osb[:, t * KOUT:(t + 1) * KOUT],
                                     in_=v3, axis=mybir.AxisListType.X)

                # ---- prefetch next supertile
                if s + 1 < NSUPER:
                    if t == 0:
                        fetch_zz(s + 1, nc.sync)
                    if t in (1, 3, 5, 6) and s + 1 < NSUPER:
                        fetch_zz_part(s + 1, {1: 0, 3: 1, 5: 2, 6: 3}[t], nc.sync)
                    if t == 6:
                        fetch_z4_xbd(s + 1, nc.scalar)
            nc.sync.dma_start(out=OUTd[s], in_=osb)
    nc.compile()
    return nc


_CACHE = {}


def _get_module():
    if "nc" not in _CACHE:
        _CACHE["nc"] = _build_module()
    return _CACHE["nc"]


def _host_inputs(X, W1, W2, W3):
    Xf = np.asarray(X, np.float32)
    Xh = Xf.astype(FP16)
    Xhf = Xh.astype(np.float32)
    Z = (Xhf[:, I_P] * Xhf[:, J_P]).astype(FP16)        # [B, 528]
    ws = _pack_weights(W1, W2, W3)

    in_maps = []
    for core in range(NCORES):
        lo, hi = core * BLOC, (core + 1) * BLOC
        view = Z[lo:hi].reshape(NSUPER, SUPER, P, NPAIRS)     # [s,t,r,p]
        zz = (view[:, :, :, :512].reshape(NSUPER, SUPER, P, 4, 128)
              .transpose(0, 4, 1, 3, 2)                       # [s, p, t, c, r]
              .reshape(NSUPER, 128, 4096))
        z4p = (view[:, :, :, 512:].transpose(0, 3, 1, 2)      # [s, q, t, r]
               .reshape(NSUPER, 16, SP))
        xpart = (Xh[lo:hi].reshape(NSUPER, SUPER, P, D)
                 .transpose(0, 3, 1, 2).reshape(NSUPER, D, SP))
        z4 = np.concatenate([z4p, xpart], axis=1)             # [s, 48, 1024]
        xbd = (Xh[lo:hi].reshape(NSUPER, SUPER, P, D)
               .transpose(0, 2, 1, 3).reshape(NSUPER, 128, SUPER * D))
        x31 = np.ascontiguousarray(
            xbd[:, :, 31::D].astype(np.float32))          # [s, 128, 8]
        m = {
            "ZZ": np.ascontiguousarray(zz),
            "Z4": np.ascontiguousarray(z4),
            "XBD": np.ascontiguousarray(xbd),
            "X31": x31,
            "W4": ws[4],
        }
        for c in range(4):
            m[f"W{c}"] = ws[c]
        in_maps.append(m)
    return in_maps


def kernel(X, W1, W2, W3, bias):
    bias = np.asarray(bias, np.float32)
    in_maps = _host_inputs(X, W1, W2, W3)
    nc = _get_module()
    res = bass_utils.run_bass_kernel_spmd(nc, in_maps, core_ids=list(range(NCORES)))
    _CACHE["last_results"] = res
    outs = []
    for c in range(NCORES):
        od = np.asarray(res.results[c]["OUT"])       # [NSUPER, 128, SUPER*KOUT]
        outs.append(od.reshape(NSUPER, P, SUPER, KOUT)
                    .transpose(0, 2, 1, 3).reshape(BLOC, KOUT))
    out = np.concatenate(outs, 0)
    return (out + bias.reshape(1, KOUT)).astype(np.float32)


# revision 9
# speedup vs baseline: 1.4195x; 1.1143x over previous
"""Trainium2 Bass kernel for NeuralTensorLayer (order-1/2/3 polynomial layer).

    out[b,l] = bias[l] + sum_i X[b,i] W1[i,l]
             + sum_ij X[b,i] X[b,j] W2[i,j,l]
             + sum_ijk X[b,i] X[b,j] X[b,k] W3[i,j,k,l]

B=32768, D=K=32, data-parallel over 8 NeuronCores (4096 rows each).

v3 strategy:
  * Full (i,j,k) symmetrization: out3 = sum_{a<=b} Z_ab sum_{c>=b} X_c
    W3f[(a,b),c,l] with W3f summing all distinct permutations of the
    sorted triple.  Pairs sorted by b make W3f's k-support a suffix
    [j0_chunk, 32) per 128-pair chunk (widths [32,17,10,5,1]): only the
    5984 unique triples are streamed instead of 16896 MACs.
  * k-split at c*=8: triples whose third factor index is < 8
    (120 (pair,k) combos) are precomputed on host as triple products
    Y[b,(p,k)] = Z_p X_k and contracted in the 32-col "low" matmul, so
    the big PSUM grid is [32 l x 24 k] (cols k=8..31, l-major, holes in
    a [128,1024] 2-bank tile).  This shrinks every stage-2 op by 25%.
  * Chunk 4's 16 pairs (j=31) contribute k=31 only - two 16-col matmuls
    straight into the big grid.
  * Z pair products and Y triples are precomputed on host (fp16) and
    DMA'd in - no on-device pair building.
  * Matmul operands fp16 (10-bit mantissa); stage-2 tensors bf16 (the
    DVE 2x packed mode exists for bf16 only).
  * Stage-2 split: ScalarE does the strided PSUM->SBUF compact copy
    (l,k>=8 -> [128,768]), DVE does the X_k broadcast multiply, the low
    copy, and the 13-wide reduce; GpSimd does the 24->12 fold add.
    Everything lands at ~1.0-1.15us/tile against PE ~1.0us/tile.
  * PSUM: big [128,1024] x3 + low [128,32] x2 = 8 banks.
"""

import numpy as np
import ml_dtypes
from contextlib import ExitStack

import concourse.bass as bass
import concourse.bacc as bacc
import concourse.tile as tile
from concourse import mybir
from concourse import bass_utils

FP16 = np.float16
BF16 = ml_dtypes.bfloat16

B, D, KOUT = 32768, 32, 32
NCORES = 8
BLOC = B // NCORES          # 4096 rows per core
P = 128                     # rows per tile
SUPER = 8                   # tiles per supertile
SP = SUPER * P              # 1024
NSUPER = BLOC // SP         # 4
NDUMMY = 4                  # PE warm-up matmuls (HAM un-throttle)
CSTAR = 8                   # k-columns below this go through Y-expansion
GRID = D - CSTAR            # 24 k-columns in the big grid
NV = GRID // 2 + 1          # v width per l: 12 folds + low

# pairs (i,j), i<=j, sorted by j then i: p = j(j+1)/2 + i
PAIRS = [(i, j) for j in range(D) for i in range(j + 1)]
NPAIRS = len(PAIRS)         # 528
I_P = np.array([p[0] for p in PAIRS], np.int64)
J_P = np.array([p[1] for p in PAIRS], np.int64)
J0 = [int(J_P[128 * c]) for c in range(4)]      # [0, 15, 22, 27]
W_C = [D - max(j, CSTAR) for j in J0]           # big widths [24,17,10,5]
JW = [max(j, CSTAR) for j in J0]                # big window starts [8,15,22,27]

# Y-expansion rows: (pair, k) for k < CSTAR, j(pair) <= k, sorted by (k, p)
YROWS = [(p, k) for k in range(CSTAR) for p in range(NPAIRS) if J_P[p] <= k]
NY = len(YROWS)             # 120
NY_A = 128 - 16 - D         # 80 Y rows in chunk 4a
NY_B = NY - NY_A            # 40 Y rows in chunk 4b

F32 = mybir.dt.float32
F16 = mybir.dt.float16
BF = mybir.dt.bfloat16


def _symmetrize(W1, W2, W3):
    W1 = np.asarray(W1, np.float64)
    W2 = np.asarray(W2, np.float64)
    W3 = np.asarray(W3, np.float64)
    from itertools import permutations
    S6 = np.zeros((D, D, D, KOUT))
    for perm in set(permutations((0, 1, 2))):
        S6 += np.transpose(W3, perm + (3,))
    W3f = np.zeros((NPAIRS, D, KOUT))
    for p, (a, bb) in enumerate(PAIRS):
        for c in range(bb, D):
            if a == bb == c:
                f = 1.0 / 6.0
            elif a == bb or bb == c:
                f = 0.5
            else:
                f = 1.0
            W3f[p, c] = S6[a, bb, c] * f
    W2s = np.empty((NPAIRS, KOUT))
    for p, (a, bb) in enumerate(PAIRS):
        W2s[p] = W2[a, bb] + W2[bb, a] if a < bb else W2[a, a]
    return W1, W2s, W3f


def _pack_weights(W1, W2, W3):
    """w0..w3: [128, 32*w + 32] fp16; w4a: [128, 64]; w4b: [40, 32]."""
    W1, W2s, W3f = _symmetrize(W1, W2, W3)
    ws = []
    for c in range(4):
        j0, w = JW[c], W_C[c]
        wt = np.zeros((128, 32 * w + 32))
        blk = W3f[128 * c:128 * (c + 1), j0:, :]        # [128, w(k), 32(l)]
        blk = np.transpose(blk, (0, 2, 1)).reshape(128, KOUT * w)  # (l,k)
        wt[:, :16 * w] = blk[:, :16 * w]
        wt[:, 16 * w:32 * w] = blk[:, 16 * w:]
        wt[:, 32 * w:] = W2s[128 * c:128 * (c + 1)]
        ws.append(wt.astype(np.float32).astype(FP16))
    # chunk 4a: 16 pairs + 32 W1 rows + 80 Y rows; low cols 0:32, k31 cols 32:64
    w4a = np.zeros((128, 64))
    w4a[:16, :KOUT] = W2s[512:]
    w4a[16:48, :KOUT] = W1
    for r, (p, k) in enumerate(YROWS[:NY_A]):
        w4a[48 + r, :KOUT] = W3f[p, k, :]
    w4a[:16, KOUT:] = W3f[512:, 31, :]
    ws.append(w4a.astype(np.float32).astype(FP16))
    w4b = np.zeros((NY_B, KOUT))
    for r, (p, k) in enumerate(YROWS[NY_A:]):
        w4b[r] = W3f[p, k, :]
    ws.append(w4b.astype(np.float32).astype(FP16))
    return ws


def _build_module():
    nc = bacc.Bacc("TRN2", target_bir_lowering=False, debug=False,
                   enable_asserts=False)
    ZZd = nc.dram_tensor("ZZ", [NSUPER, 128, 8 * 512], F16, kind="ExternalInput").ap()
    Z4Ad = nc.dram_tensor("Z4A", [NSUPER, 128, SP], F16, kind="ExternalInput").ap()
    Z4Bd = nc.dram_tensor("Z4B", [NSUPER, NY_B, SP], F16, kind="ExternalInput").ap()
    XBDd = nc.dram_tensor("XBD", [NSUPER, 128, SUPER * GRID], BF, kind="ExternalInput").ap()
    Wd = [nc.dram_tensor(f"W{c}", [128, 32 * W_C[c] + 32], F16,
                         kind="ExternalInput").ap()
          for c in range(4)]
    W4Ad = nc.dram_tensor("W4A", [128, 64], F16, kind="ExternalInput").ap()
    W4Bd = nc.dram_tensor("W4B", [NY_B, KOUT], F16, kind="ExternalInput").ap()
    OUTd = nc.dram_tensor("OUT", [NSUPER, 128, SUPER * KOUT], F32, kind="ExternalOutput").ap()

    with ExitStack() as ctx:
        tc = ctx.enter_context(tile.TileContext(nc))
        consts = ctx.enter_context(tc.tile_pool(name="consts", bufs=1))
        zzpool = ctx.enter_context(tc.tile_pool(name="zzpool", bufs=2))
        z4pool = ctx.enter_context(tc.tile_pool(name="z4pool", bufs=2))
        xbpool = ctx.enter_context(tc.tile_pool(name="xbpool", bufs=2))
        spool = ctx.enter_context(tc.tile_pool(name="spool", bufs=4))
        upool = ctx.enter_context(tc.tile_pool(name="upool", bufs=4))
        vpool = ctx.enter_context(tc.tile_pool(name="vpool", bufs=4))
        opool = ctx.enter_context(tc.tile_pool(name="opool", bufs=2))
        bigps = ctx.enter_context(tc.tile_pool(name="bigps", bufs=3, space="PSUM"))
        lowps = ctx.enter_context(tc.tile_pool(name="lowps", bufs=2, space="PSUM"))

        g = consts.tile([128, 640], F16, tag="g")
        nc.vector.memset(g, 0.0)

        w_sb = [consts.tile([128, 32 * W_C[c] + 32], F16, tag=f"w_{c}",
                            name=f"w_{c}")
                for c in range(4)]
        w4a_sb = consts.tile([128, 64], F16, tag="w4a")
        w4b_sb = consts.tile([NY_B, KOUT], F16, tag="w4b")

        zz = {}
        z4a = {}
        z4b = {}
        xbd = {}

        def fetch_super(s, eng):
            """zz: one DMA for s>0, split for startup pipelining."""
            zt = zzpool.tile([128, 8 * 512], F16, tag="zz", name=f"zz{s}")
            zz[s] = zt
            if s == 0:
                eng.dma_start(out=zt[:, 0:512], in_=ZZd[0][:, 0:512])
                eng.dma_start(out=zt[:, 512:1024], in_=ZZd[0][:, 512:1024])
                eng.dma_start(out=zt[:, 1024:2048], in_=ZZd[0][:, 1024:2048])
                eng.dma_start(out=zt[:, 2048:4096], in_=ZZd[0][:, 2048:4096])
            else:
                eng.dma_start(out=zt, in_=ZZd[s])

        def fetch_aux(s, eng):
            at = z4pool.tile([128, SP], F16, tag="z4a", name=f"z4a_{s}")
            z4a[s] = at
            eng.dma_start(out=at, in_=Z4Ad[s])
            bt = z4pool.tile([NY_B, SP], F16, tag="z4b", name=f"z4b_{s}")
            z4b[s] = bt
            eng.dma_start(out=bt, in_=Z4Bd[s])
            xt = xbpool.tile([128, SUPER * GRID], BF, tag="xbd", name=f"xbd{s}")
            xbd[s] = xt
            eng.dma_start(out=xt, in_=XBDd[s])

        # ---- startup DMAs
        nc.scalar.dma_start(out=w_sb[0], in_=Wd[0])
        fetch_super(0, nc.sync)
        nc.scalar.dma_start(out=w_sb[1], in_=Wd[1])
        nc.gpsimd.dma_start(out=w4a_sb, in_=W4Ad)
        nc.gpsimd.dma_start(out=w4b_sb, in_=W4Bd)
        fetch_aux(0, nc.gpsimd)
        nc.scalar.dma_start(out=w_sb[2], in_=Wd[2])
        nc.scalar.dma_start(out=w_sb[3], in_=Wd[3])

        # PE warm-up (results discarded; tiles recycled by the pool)
        for _ in range(NDUMMY):
            dummy = bigps.tile([128, 1024], F32, tag="big")
            nc.tensor.matmul(dummy[:, 0:512], g[:, :128], g[:, 128:640],
                             start=True, stop=True)

        for s in range(NSUPER):
            osb = opool.tile([128, SUPER * KOUT], F32, tag="osb")
            for t in range(SUPER):
                big = bigps.tile([128, 1024], F32, tag="big")
                low = lowps.tile([128, 32], F32, tag="low")
                bigv = big[:, :].rearrange("p (l k) -> p l k", k=D)
                # chunks 0-3: suffix k windows, strided PSUM writes
                for c in range(4):
                    j0, w = JW[c], W_C[c]
                    zc = zz[s][:, t * 512 + c * 128: t * 512 + (c + 1) * 128]
                    first = c == 0
                    nc.tensor.matmul(bigv[:, 0:16, j0:D], zc, w_sb[c][:, 0:16 * w],
                                     start=first, stop=False)
                    nc.tensor.matmul(bigv[:, 16:32, j0:D], zc, w_sb[c][:, 16 * w:32 * w],
                                     start=first, stop=False)
                    nc.tensor.matmul(low, zc, w_sb[c][:, 32 * w:32 * w + 32],
                                     start=first, stop=False)
                # chunk 4a: 16 pairs (k=31 into big) + W1 rows + 80 Y rows (low)
                za = z4a[s][:, t * 128:(t + 1) * 128]
                nc.tensor.matmul(bigv[:, 0:16, 31:32], za[0:16, :],
                                 w4a_sb[0:16, 32:48], start=False, stop=True)
                nc.tensor.matmul(bigv[:, 16:32, 31:32], za[0:16, :],
                                 w4a_sb[0:16, 48:64], start=False, stop=True)
                nc.tensor.matmul(low, za, w4a_sb[:, 0:32],
                                 start=False, stop=False)
                # chunk 4b: 40 more Y rows (low)
                zb = z4b[s][:, t * 128:(t + 1) * 128]
                nc.tensor.matmul(low, zb, w4b_sb,
                                 start=False, stop=True)

                # ---- stage 2
                staged = spool.tile([128, KOUT * GRID], BF, tag="staged")
                st3 = staged[:, :].rearrange("p (l k) -> p l k", k=GRID)
                nc.scalar.copy(out=st3, in_=bigv[:, :, CSTAR:D])
                u = upool.tile([128, KOUT * GRID], BF, tag="u")
                u3 = u[:, :].rearrange("p (l k) -> p l k", k=GRID)
                xk = (xbd[s][:, t * GRID:(t + 1) * GRID]
                      .unsqueeze(1).broadcast_to([P, KOUT, GRID]))
                nc.vector.tensor_mul(u3, st3, xk)
                v = vpool.tile([128, KOUT * NV], BF, tag="v")
                v3 = v[:, :].rearrange("p (l c) -> p l c", c=NV)
                nc.gpsimd.tensor_add(v3[:, :, 0:GRID // 2], u3[:, :, 0:GRID // 2],
                                     u3[:, :, GRID // 2:GRID])
                nc.vector.tensor_copy(v3[:, :, GRID // 2:NV], low[:, :].unsqueeze(2))
                nc.vector.reduce_sum(out=osb[:, t * KOUT:(t + 1) * KOUT],
                                     in_=v3, axis=mybir.AxisListType.X)

                # ---- prefetch next supertile
                if s + 1 < NSUPER:
                    if t == 0:
                        fetch_super(s + 1, nc.sync)
                    if t == 5:
                        fetch_aux(s + 1, nc.scalar)
            nc.sync.dma_start(out=OUTd[s], in_=osb)
    nc.compile()
    return nc


_CACHE = {}


def _get_module():
    if "nc" not in _CACHE:
        _CACHE["nc"] = _build_module()
    return _CACHE["nc"]


def _host_inputs(X, W1, W2, W3):
    Xf = np.asarray(X, np.float32)
    Xh = Xf.astype(FP16)
    Xhf = Xh.astype(np.float32)
    Z = (Xhf[:, I_P] * Xhf[:, J_P]).astype(FP16)        # [B, 528]
    YP = np.array([r[0] for r in YROWS])
    YK = np.array([r[1] for r in YROWS])
    Y = (Z[:, YP].astype(np.float32) * Xhf[:, YK]).astype(FP16)  # [B, 120]
    Xb = Xf.astype(BF16)
    ws = _pack_weights(W1, W2, W3)

    in_maps = []
    for core in range(NCORES):
        lo, hi = core * BLOC, (core + 1) * BLOC
        view = Z[lo:hi].reshape(NSUPER, SUPER, P, NPAIRS)     # [s,t,r,p]
        zz = (view[:, :, :, :512].reshape(NSUPER, SUPER, P, 4, 128)
              .transpose(0, 4, 1, 3, 2)                       # [s, p, t, c, r]
              .reshape(NSUPER, 128, 4096))
        yv = Y[lo:hi].reshape(NSUPER, SUPER, P, NY)           # [s,t,r,y]
        # z4a rows: 16 pairs | 32 X | 80 Y ; z4b rows: 40 Y
        z4a = np.empty((NSUPER, 128, SP), FP16)
        z4a[:, 0:16] = view[:, :, :, 512:].transpose(0, 3, 1, 2).reshape(NSUPER, 16, SP)
        z4a[:, 16:48] = (Xh[lo:hi].reshape(NSUPER, SUPER, P, D)
                         .transpose(0, 3, 1, 2).reshape(NSUPER, D, SP))
        z4a[:, 48:128] = yv[:, :, :, :NY_A].transpose(0, 3, 1, 2).reshape(NSUPER, NY_A, SP)
        z4b = np.ascontiguousarray(
            yv[:, :, :, NY_A:].transpose(0, 3, 1, 2).reshape(NSUPER, NY_B, SP))
        xbd = (Xb[lo:hi, CSTAR:].reshape(NSUPER, SUPER, P, GRID)
               .transpose(0, 2, 1, 3).reshape(NSUPER, 128, SUPER * GRID))
        m = {
            "ZZ": np.ascontiguousarray(zz),
            "Z4A": z4a,
            "Z4B": z4b,
            "XBD": np.ascontiguousarray(xbd),
            "W4A": ws[4],
            "W4B": ws[5],
        }
        for c in range(4):
            m[f"W{c}"] = ws[c]
        in_maps.append(m)
    return in_maps


def kernel(X, W1, W2, W3, bias):
    bias = np.asarray(bias, np.float32)
    in_maps = _host_inputs(X, W1, W2, W3)
    nc = _get_module()
    res = bass_utils.run_bass_kernel_spmd(nc, in_maps, core_ids=list(range(NCORES)))
    _CACHE["last_results"] = res
    outs = []
    for c in range(NCORES):
        od = np.asarray(res.results[c]["OUT"])       # [NSUPER, 128, SUPER*KOUT]
        outs.append(od.reshape(NSUPER, P, SUPER, KOUT)
                    .transpose(0, 2, 1, 3).reshape(BLOC, KOUT))
    out = np.concatenate(outs, 0)
    return (out + bias.reshape(1, KOUT)).astype(np.float32)
